# revision 3
# baseline (speedup 1.0000x reference)
"""Trainium2 Bass kernel for nn_HNN_layer (dense_mlp, memory regime), v2.

Math: the reference never increments start_i, so every block reads
x[:, 0:fn] with fn <= 13; the module collapses to

    out = sigmoid(relu(x[:, :13] @ W + b) @ fk + fb)          (B, 1)

Device strategy (8 cores data-parallel, Bc = 131072 rows/core, padded to
52 bursts x 5 groups x 512 cols):
  - MM1 (plain fp8, K=70, block-diagonal 5-group weights): serial 512-col
    matmuls; plain fp8 keeps the HAM clock-gate open (DoubleRow does not)
    so they stream at ~216 ns each (2.4 GHz).
  - Exit: relu(h)*(1/16) PSUM->SBUF bf16 per pair (110, 1024), alternating
    ScalarE activation / VectorE tensor_scalar.
  - MM2 (bf16, (110,32) weights): 4 bursts per macro issued together at
    tile_position (0, 32c) -> 4-way col-group co-streaming into one
    (128, 512) PSUM bank.
  - sigmoid(z + fb) on ScalarE per macro, f32 into osb.
  - Output: compacted DMA of only the 20 useful partitions per burst-slot
    (5 groups x 4 col-slots), one DMA per 4-macro group on the Sync queue.
"""

import sys

if "/opt/trn_rl_repo" not in sys.path:
    sys.path.insert(0, "/opt/trn_rl_repo")

from contextlib import ExitStack

import numpy as np
import ml_dtypes

import concourse.bass as bass
import concourse.bacc as bacc
import concourse.mybir as mybir
import concourse.tile as tile
from concourse.bass_utils import run_bass_kernel_spmd

FEATURE_LIST = [10, 13, 13, 7, 3, 6, 3, 13, 5, 4, 6, 4, 5, 4, 4, 5, 4, 3, 3, 7, 3, 3]
NB = len(FEATURE_LIST)   # 22 blocks
FMAX = 13                # only x[:, :13] is ever read
B_TOTAL = 1048576
N_CORES = 8
BC = B_TOTAL // N_CORES  # 131072 rows per core
G = 5                    # batch groups packed per matmul column
N = 512                  # free-dim columns per burst
NBURST = 52              # bursts per core (padded)
Q = NBURST * N           # 26624 padded rows per group
NMAC = NBURST // 4       # 13 macros of 4 bursts
KA = FMAX + 1            # 14 = features + ones row (bias fold)
KP = KA * G              # 70 contraction partitions for plain-fp8 MM1
MP = NB * G              # 110 h partitions
W1SCALE = 16.0           # pow2 prescale keeps fp8 W1 out of subnormals
XCH = [4, 8, 18, 22]     # x chunks in bursts
OB = 4                   # macros per osb tile / output DMA group

BF16 = mybir.dt.bfloat16
FP8 = mybir.dt.float8e4
F32 = mybir.dt.float32

USE_BLOCK_AP = False  # single-trigger DMAs with 2-level partition patterns

_BUILD_CACHE = {}


def _exit_engine_flags():
    """Per-pair exit engine: True=VectorE, False=ScalarE(ACT).

    2 exits per macro; ACT also runs 1 sigmoid per macro (512 cols) and
    is faster per column (0.83 vs 1.04 ns), so give ACT ~45% of exits.
    """
    n = 2 * NMAC
    return [u % 3 != 2 for u in range(n)]


def build_program():
    nc = bacc.Bacc("TRN2", target_bir_lowering=False, debug=False)

    # x: [14g+k, t*512 + n] = x_aug[k] of (group g, burst t, col n)
    xg = nc.dram_tensor("xg", [KP, NBURST * N], FP8,
                        kind="ExternalInput").ap()
    w1d = nc.dram_tensor("w1d", [KP, MP], FP8, kind="ExternalInput").ap()
    w2d = nc.dram_tensor("w2d", [MP, 32], BF16, kind="ExternalInput").ap()
    cst = nc.dram_tensor("cst", [128, 2], F32, kind="ExternalInput").ap()
    # out row 5c+g, col m*512+n  <->  sigmoid(burst 4m+c, group g, pos n)
    # (macros 0..10; macros 11-12 go to outp uncompacted: one fast trigger)
    outd = nc.dram_tensor("out", [4 * G, (NMAC - 2) * N], F32,
                          kind="ExternalOutput").ap()
    outp = nc.dram_tensor("outp", [128, 2 * N], F32,
                          kind="ExternalOutput").ap()

    dve_flags = _exit_engine_flags()

    with tile.TileContext(nc) as tc, ExitStack() as ctx:
        const = ctx.enter_context(tc.tile_pool(name="const", bufs=1))
        hps_pool = ctx.enter_context(
            tc.tile_pool(name="hps", bufs=3, space="PSUM"))
        zps_pool = ctx.enter_context(
            tc.tile_pool(name="zps", bufs=2, space="PSUM"))
        hsb_pool = ctx.enter_context(tc.tile_pool(name="hsb", bufs=8))
        osb_pool = ctx.enter_context(tc.tile_pool(name="osb", bufs=2))

        # x chunk DMAs stream on the Sync queue; consts go via ScalarE
        # (also a HWDGE engine) so the x path isn't serialized behind them.
        cst_t = const.tile([128, 2], F32)
        nc.sync.dma_start(cst_t[:], cst[:])
        fbv_ap = cst_t[:, 1:2]

        xch_tiles = []
        xch_start = []  # first burst of each chunk
        t0 = 0
        for ci, nt in enumerate(XCH):
            xt = const.tile([KP, nt * N], FP8, name=f"xc{ci}")
            nc.sync.dma_start(xt[:], xg[:, t0 * N:(t0 + nt) * N])
            xch_tiles.append(xt)
            xch_start.append(t0)
            t0 += nt

        w1_t = const.tile([KP, MP], FP8)
        nc.gpsimd.dma_start(w1_t[:], w1d[:])
        w2_t = const.tile([MP, 32], BF16)
        nc.gpsimd.dma_start(w2_t[:], w2d[:])

        # ACT warmup: trigger the activation table loads during the DMA
        # wait, off a memset tile so no DMA dependency delays them.
        wf32 = const.tile([128, 1], F32)
        nc.vector.memset(wf32[:], 0.5)
        warm = const.tile([128, 1], F32)
        nc.scalar.activation(
            warm[:], wf32[:], mybir.ActivationFunctionType.Sigmoid)
        nc.scalar.activation(
            warm[:], wf32[:], mybir.ActivationFunctionType.Relu)

        # PE warmup: 17 dep-free dummy matmuls (~7us cold) open the HAM
        # clock-gate (needs one fully-busy 4096-cycle window; >=2 windows
        # of gapless work guarantees coverage regardless of phase). The
        # wdum/rdum tiles also feed the per-macro filler matmuls that keep
        # the gate open in steady state.
        wdum = const.tile([110, 32], BF16)
        rdum = const.tile([110, N], BF16)
        nc.vector.memset(wdum[:], 0.25)
        nc.vector.memset(rdum[:], 0.25)
        wps = zps_pool.tile([128, N], F32, name="warmps", tag="zps")
        for _ in range(17):
            nc.tensor.matmul(
                wps[0:32, :], wdum[:], rdum[:], start=True, stop=True)

        def x_ap(t):
            """rhs AP (70, 512) for burst t."""
            ci = 0
            for k in range(len(XCH)):
                if xch_start[k] <= t < xch_start[k] + XCH[k]:
                    ci = k
                    break
            tl = t - xch_start[ci]
            return xch_tiles[ci][:, tl * N:(tl + 1) * N]

        hps_of = {}   # pair index -> hps tile (110, 1024)
        hsb_of = {}   # pair index -> hsb tile (110, 1024)
        zps_of = {}   # macro -> zps tile
        osb_state = {"t": None}

        def emit_mm1_pair(u):
            """MM1 for bursts 2u, 2u+1 into one (110, 1024) hps tile."""
            hps = hps_pool.tile([MP, 2 * N], F32, name="hps", tag="hps")
            for r in range(2):
                t = 2 * u + r
                nc.tensor.matmul(
                    hps[:, r * N:(r + 1) * N],
                    w1_t[:], x_ap(t),
                    start=True, stop=True,
                )
            hps_of[u] = hps

        def emit_exit_pair(u):
            hps = hps_of.pop(u)
            hsb = hsb_pool.tile([MP, 2 * N], BF16, name="hsb", tag="hsb")
            if u < 4:
                # pipeline fill: split the pair across both engines so the
                # hps banks free quickly and the PE never stalls long
                nc.vector.tensor_scalar(
                    hsb[:, 0:N], hps[:, 0:N],
                    scalar1=1.0 / W1SCALE, scalar2=0.0,
                    op0=mybir.AluOpType.mult, op1=mybir.AluOpType.max,
                )
                nc.scalar.activation(
                    hsb[:, N:2 * N], hps[:, N:2 * N],
                    mybir.ActivationFunctionType.Relu,
                    scale=1.0 / W1SCALE,
                )
            elif dve_flags[u]:
                nc.vector.tensor_scalar(
                    hsb[:], hps[:],
                    scalar1=1.0 / W1SCALE, scalar2=0.0,
                    op0=mybir.AluOpType.mult, op1=mybir.AluOpType.max,
                )
            else:
                nc.scalar.activation(
                    hsb[:], hps[:],
                    mybir.ActivationFunctionType.Relu,
                    scale=1.0 / W1SCALE,
                )
            hsb_of[u] = hsb

        def emit_mm2_group(m):
            """Four co-streaming MM2s (col groups) for macro m."""
            zps = zps_pool.tile([128, N], F32, name="zps", tag="zps")
            zps_of[m] = zps
            # HAM filler: the exits pace the pipeline slightly above the
            # PE's warm-clock work, and the clock-gate re-throttles on idle
            # windows; burn the slack on a dummy matmul whose output the
            # real c=0 matmul overwrites (start=True).
            nc.tensor.matmul(
                zps[0:32, 0:N // 2], wdum[:], rdum[:, 0:N // 2],
                start=True, stop=True)
            for c in range(4):
                hsb = hsb_of[2 * m + c // 2]
                nc.tensor.matmul(
                    zps[32 * c:32 * c + 32, :],
                    w2_t[:], hsb[:, (c % 2) * N:(c % 2 + 1) * N],
                    start=True, stop=True,
                    tile_position=(0, 32 * c),
                )
            del hsb_of[2 * m]
            del hsb_of[2 * m + 1]

        # output groups: macros [0-3], [4-7], [8-10] compacted; [11-12]
        # uncompacted (one trigger right after the last sigmoid)
        OGRP = {0: 4, 4: 4, 8: 3, 11: 2}

        def emit_sigmoid_out(m):
            zps = zps_of.pop(m)
            if m in OGRP:
                osb_state["t"] = osb_pool.tile(
                    [128, OGRP[m] * N], F32, name="osb", tag="osb")
                osb_state["m0"] = m
            osb = osb_state["t"]
            m0 = osb_state["m0"]
            mo = m - m0
            nc.scalar.activation(
                osb[:, mo * N:(mo + 1) * N], zps[:],
                mybir.ActivationFunctionType.Sigmoid,
                bias=fbv_ap,
            )
            if m - m0 == OGRP[m0] - 1:
                if m0 == 11:
                    nc.sync.dma_start(outp[:], osb[:])
                else:
                    for c in range(4):
                        nc.sync.dma_start(
                            outd[5 * c:5 * c + G, m0 * N:(m + 1) * N],
                            osb[32 * c:32 * c + G, :],
                        )

        # Software pipeline: MM2 groups trail MM1 by 2 macros so the
        # in-order PE queue never waits on an exit.
        for u in range(4):
            emit_mm1_pair(u)
            emit_exit_pair(u)
        for m in range(2, NMAC):
            emit_mm1_pair(2 * m)
            emit_mm2_group(m - 2)
            emit_exit_pair(2 * m)
            emit_mm1_pair(2 * m + 1)
            emit_sigmoid_out(m - 2)
            emit_exit_pair(2 * m + 1)
        for m in (NMAC - 2, NMAC - 1):
            emit_mm2_group(m)
            emit_sigmoid_out(m)

    nc.compile()
    return nc


def _pack_host_inputs(x, kernels, biases, final_kernel):
    """Build per-core device arrays from the full inputs."""
    W = np.zeros((FMAX, NB), np.float32)
    off = 0
    for i, fn in enumerate(FEATURE_LIST):
        W[:fn, i] = np.asarray(kernels[off:off + fn, 0], np.float32)
        off += fn
    b = np.asarray(biases, np.float32)
    fk = np.asarray(final_kernel[:, 0], np.float32)

    w2 = np.zeros((MP, 32), np.float32)
    cst = np.zeros((128, 2), np.float32)
    for g in range(G):
        w2[NB * g:NB * (g + 1), g] = fk
        cst[NB * g:NB * (g + 1), 0] = b
    w2 = w2.astype(ml_dtypes.bfloat16)

    # W_aug rows 0..12 = W * SCALE, row 13 = b * SCALE
    Wa = np.zeros((KA, NB), np.float32)
    Wa[:FMAX] = W * W1SCALE
    Wa[FMAX] = b * W1SCALE
    w1 = np.zeros((KP, MP), np.float32)
    for g in range(G):
        w1[KA * g:KA * (g + 1), NB * g:NB * (g + 1)] = Wa
    w1 = w1.astype(ml_dtypes.float8_e4m3)

    x13 = np.ascontiguousarray(np.asarray(x[:, :FMAX], np.float32)).astype(
        ml_dtypes.float8_e4m3)
    one = ml_dtypes.float8_e4m3(1.0)
    in_maps = []
    for cidx in range(N_CORES):
        # X[14g+k, t*512+n] = x_aug[k, g*Q + t*512 + n]
        X = np.zeros((KP, NBURST * N), ml_dtypes.float8_e4m3)
        base = cidx * BC
        for g in range(G):
            v = min(Q, BC - g * Q)
            xa = np.full((KA, Q), one)
            xa[:FMAX, :v] = x13[base + g * Q:base + g * Q + v, :].T
            if v < Q:
                xa[:FMAX, v:] = 0
            X[KA * g:KA * (g + 1)] = xa
        in_maps.append({"xg": X, "w1d": w1, "w2d": w2, "cst": cst})
    return in_maps


def run(x, kernels, biases, final_kernel, final_bias, trace=False,
        **spmd_kwargs):
    if "nc" not in _BUILD_CACHE:
        _BUILD_CACHE["nc"] = build_program()
    nc = _BUILD_CACHE["nc"]

    fb = float(np.asarray(final_bias).reshape(-1)[0])
    in_maps = _pack_host_inputs(x, kernels, biases, final_kernel)
    for m in in_maps:
        m["cst"][:, 1] = fb
    res = run_bass_kernel_spmd(
        nc, in_maps, list(range(N_CORES)), trace=trace, **spmd_kwargs)
    outs = []
    for cidx in range(N_CORES):
        op = np.asarray(res.results[cidx]["out"], np.float32)  # (20, 11*512)
        opp = np.asarray(res.results[cidx]["outp"], np.float32)  # (128, 1024)
        arr = np.empty((G, NMAC, 4, N), np.float32)
        arr[:, :NMAC - 2] = op.reshape(4, G, NMAC - 2, N).transpose(1, 2, 0, 3)
        arr[:, NMAC - 2:] = opp.reshape(4, 32, 2, N)[:, :G].transpose(
            1, 2, 0, 3)
        outs.append(arr.reshape(G, Q).reshape(-1)[:BC])
    y = np.concatenate(outs).reshape(B_TOTAL, 1)
    return y, res


def kernel(x, kernels, biases, final_kernel, final_bias):
    y, _ = run(x, kernels, biases, final_kernel, final_bias, trace=False)
    return y
